# revision 2
# baseline (speedup 1.0000x reference)
"""GCN (2-layer) Trainium2 kernel over 8 NeuronCores.

Strategy:
- Nodes sharded round-robin-contiguous: core i owns nodes [6250*i, 6250*(i+1)).
- h1 = (x @ W1) scaled by dinv (deg^-1/2) computed shard-local -> AllGather
  to a full 50176-row table in each core's DRAM.
- Scatter-add aggregation out[d] += h'[src] over edges (incl self-loops) is
  done per dst-core: per-edge rows are fetched with gpsimd.dma_gather (int16
  indices, so the table is addressed as two 25088-row halves), and the
  segmented sum over each 128-dst tile is a TensorE matmul with a DVE-built
  one-hot selector.
- Layer 2 aggregates the dinv-scaled relu(out1) table (64 features) first,
  then applies W2 + bias + log_softmax on-chip.
The edge structure is baked into the program at build time (the SPMD program
is identical on all 8 cores; per-core data differs, padded to common shape).
"""

import numpy as np

N_NODES = 50000
CORES = 8
SH = 6250          # owned nodes per core
SHP = 6272         # padded shard rows (49*128)
NT = 49            # dst tiles per core
HALF = SHP * 4     # 25088 table rows per half
F0, F1, F2 = 96, 64, 16
BLK = 128
CHUNK_BLOCKS = 8   # 1024 idx per dma_gather (single_packet limit)
CHUNK = BLK * CHUNK_BLOCKS


def _row_of_node(n):
    s = n // SH
    return s * SHP + (n - s * SH)


def host_prep(x, edge_index, W1, b1, W2, b2):
    """Build all per-core arrays + the uniform program structure."""
    src = np.asarray(edge_index[0], dtype=np.int64)
    dst = np.asarray(edge_index[1], dtype=np.int64)

    deg_full = np.bincount(dst, minlength=N_NODES).astype(np.float32) + 1.0

    # per-core edge lists (dst-sharded), with self-loops appended
    per_core = []
    order = np.argsort(dst, kind="stable")
    s_sorted, d_sorted = src[order], dst[order]
    bounds = np.searchsorted(d_sorted, np.arange(0, N_NODES + 1, SH))
    for i in range(CORES):
        es = s_sorted[bounds[i]:bounds[i + 1]]
        ed = d_sorted[bounds[i]:bounds[i + 1]]
        loops = np.arange(SH * i, SH * (i + 1), dtype=np.int64)
        es = np.concatenate([es, loops])
        ed = np.concatenate([ed, loops]) - SH * i  # local dst [0, 6250)
        per_core.append((es, ed))

    # split per (core, tile, half); gather row indices (half-local)
    ZROW = [SH, SH]  # zero row local idx within each half (pad rows are zero)
    runs = [[[None, None] for _ in range(NT)] for _ in range(CORES)]
    for i in range(CORES):
        es, ed = per_core[i]
        rows = _row_of_node(es)
        half = (rows >= HALF).astype(np.int64)
        lrow = rows - half * HALF
        tile = ed // BLK
        dl = ed - tile * BLK
        key = tile * 2 + half
        o = np.argsort(key, kind="stable")
        key_s, lrow_s, dl_s = key[o], lrow[o], dl[o]
        kb = np.searchsorted(key_s, np.arange(NT * 2 + 1))
        for t in range(NT):
            for h in (0, 1):
                a, b = kb[t * 2 + h], kb[t * 2 + h + 1]
                runs[i][t][h] = (lrow_s[a:b], dl_s[a:b])

    # uniform block counts
    B = np.zeros((NT, 2), dtype=np.int64)
    for t in range(NT):
        for h in (0, 1):
            mx = max(len(runs[i][t][h][0]) for i in range(CORES))
            B[t, h] = max(1, -(-mx // BLK))
    nblocks = [int(B[:, h].sum()) for h in (0, 1)]
    tile_mech = np.zeros(NT, dtype=np.int64)  # all ant-gather
    # ant stream position per (h, gb); ind col per (h, gb)
    ant_pos = [dict(), dict()]
    ind_pos = [dict(), dict()]
    nant = [0, 0]
    nind = 0
    # block gb -> (chunk, slot) implicit; (t, h) -> start block
    startgb = np.zeros((NT, 2), dtype=np.int64)
    acc = [0, 0]
    for t in range(NT):
        for h in (0, 1):
            startgb[t, h] = acc[h]
            for b in range(int(B[t, h])):
                gb = acc[h] + b
                if tile_mech[t]:
                    ind_pos[h][gb] = nind
                    nind += 1
                else:
                    ant_pos[h][gb] = nant[h]
                    nant[h] += 1
            acc[h] += B[t, h]
    nchunks = [-(-max(n, 1) // CHUNK_BLOCKS) for n in nant]

    # per-core streams
    data = []
    for i in range(CORES):
        idx_stream = [np.zeros(0, np.int64), np.zeros(0, np.int64)]
        dl_stream = [np.zeros(0, np.int64), np.zeros(0, np.int64)]
        ind_cols = np.zeros((BLK, max(nind, 1)), np.int64)
        for h in (0, 1):
            parts_i, parts_d, parts_ai = [], [], []
            for t in range(NT):
                r, d = runs[i][t][h]
                pad = int(B[t, h]) * BLK - len(r)
                ri = np.concatenate([r, np.full(pad, ZROW[h], np.int64)])
                di = np.concatenate([d, np.zeros(pad, np.int64)])
                parts_i.append(ri)
                parts_d.append(di)
                if tile_mech[t]:
                    blk = ri.reshape(-1, BLK) + h * HALF  # global rows
                    for b in range(int(B[t, h])):
                        ind_cols[:, ind_pos[h][int(startgb[t, h]) + b]] = blk[b]
                else:
                    parts_ai.append(ri)
            # dl stream covers ALL blocks (for the S plane)
            sd = np.concatenate(parts_d)
            dl_stream[h] = sd
            # ant gather stream: only ant blocks
            si = (np.concatenate(parts_ai) if parts_ai else
                  np.full(BLK, ZROW[h], np.int64))
            tail = nchunks[h] * CHUNK - len(si)
            si = np.concatenate([si, np.full(tail, ZROW[h], np.int64)])
            idx_stream[h] = si

        # int16 idx planes [128, total/16]: idx j at [j%16 + 16k, j//16]
        planes, dls = [], []
        for h in (0, 1):
            si = idx_stream[h]
            pl = si.reshape(-1, 16).T.astype(np.int16)  # [16, S/16]
            planes.append(np.tile(pl, (8, 1)))
            # dstl plane f32 [128, nblocks]: edge (gb*128+p) at [p, gb]
            dls.append(np.ascontiguousarray(
                dl_stream[h].reshape(-1, BLK).T[:, :nblocks[h]].astype(np.float32)))

        # deg plane [128, NT]: dst (t*128+p) at [p, t]; pad 1.0
        degp = np.ones((BLK, NT), np.float32)
        dshard = deg_full[SH * i:SH * (i + 1)]
        dp = np.concatenate([dshard, np.ones(SHP - SH, np.float32)])
        degp[:, :] = dp.reshape(NT, BLK).T

        # xT shard [96, 6272] zero-padded
        xs = np.zeros((F0, SHP), np.float32)
        xs[:, :SH] = np.asarray(x[SH * i:SH * (i + 1)], np.float32).T
        data.append(dict(
            xT=np.ascontiguousarray(xs),
            idx0=np.ascontiguousarray(planes[0]), idx1=np.ascontiguousarray(planes[1]),
            dl0=np.ascontiguousarray(dls[0]), dl1=np.ascontiguousarray(dls[1]),
            deg=np.ascontiguousarray(degp),
            indix=np.ascontiguousarray(ind_cols.astype(np.int32)),
        ))

    consts = dict(
        W1=np.asarray(W1, np.float32), W2=np.asarray(W2, np.float32),
        b1b=np.tile(np.asarray(b1, np.float32), (BLK, 1)),
        b2b=np.tile(np.asarray(b2, np.float32), (BLK, 1)),
        iota=np.tile(np.arange(BLK, dtype=np.float32), (BLK, 1)),
    )
    meta = dict(B=B, nblocks=nblocks, nchunks=nchunks, startgb=startgb,
                tile_mech=tile_mech, ant_pos=ant_pos, ind_pos=ind_pos,
                nind=nind, nant=nant)
    return data, consts, meta


def numpy_device_sim(data, consts, meta):
    """Replay the device algorithm in numpy (for host-side validation)."""
    B, startgb = meta["B"], meta["startgb"]
    outs = []
    # build each core's table shard then "allgather"
    tables = []
    dinvs = []
    for i in range(CORES):
        d = data[i]
        dinv = 1.0 / np.sqrt(d["deg"])  # [128, NT]
        dinvs.append(dinv)
        h = d["xT"].T @ consts["W1"]  # [6272, 64]
        hs = h.reshape(NT, BLK, F1) * dinv.T[:, :, None]
        tables.append(hs.reshape(SHP, F1))
    table = np.concatenate(tables, 0)  # [50176, 64]

    def layer(table, i, d):
        halves = [table[:HALF], table[HALF:]]
        agg = np.zeros((NT, F1, BLK), np.float32)
        for h in (0, 1):
            plane = d["idx0"] if h == 0 else d["idx1"]
            dl = d["dl0"] if h == 0 else d["dl1"]
            stream = plane[:16].T.reshape(-1)  # un-wrap
            for t in range(NT):
                for b in range(int(B[t, h])):
                    gb = int(startgb[t, h]) + b
                    rows = stream[gb * BLK:(gb + 1) * BLK].astype(np.int64)
                    G = halves[h][rows]          # [128, 64]
                    dloc = dl[:, gb].astype(np.int64)
                    S = np.zeros((BLK, BLK), np.float32)
                    S[np.arange(BLK), dloc] = 1.0
                    agg[t] += G.T @ S
        return agg  # [NT, 64, 128] (feat, dst)

    full2 = []
    for i in range(CORES):
        d = data[i]
        agg = layer(table, i, d)
        dinv = dinvs[i]
        t2 = []
        for t in range(NT):
            a = agg[t].T  # [128 dst, 64]
            e = np.maximum(a * dinv[:, t:t + 1] + consts["b1b"], 0.0) * dinv[:, t:t + 1]
            t2.append(e)
        full2.append(np.stack(t2).reshape(SHP, F1))
    table2 = np.concatenate(full2, 0)

    for i in range(CORES):
        d = data[i]
        agg = layer(table2, i, d)
        dinv = dinvs[i]
        o = np.zeros((NT, BLK, F2), np.float32)
        for t in range(NT):
            a = agg[t].T * dinv[:, t:t + 1]  # [128, 64] scaled
            z = a @ consts["W2"] + consts["b2b"]
            m = z.max(1, keepdims=True)
            ls = z - m - np.log(np.exp(z - m).sum(1, keepdims=True))
            o[t] = ls
        outs.append(o.reshape(SHP, F2))
    return np.stack(outs)  # [8, 6272, 16]


def assemble_output(outs):
    res = np.zeros((N_NODES, F2), np.float32)
    for i in range(CORES):
        res[SH * i:SH * (i + 1)] = outs[i][:SH]
    return res


def build_nc(meta):
    import concourse.bacc as bacc
    import concourse.tile as tile
    import concourse.mybir as mybir
    from concourse import bass

    dt = mybir.dt.float32
    Alu = mybir.AluOpType
    Act = mybir.ActivationFunctionType
    B, nblocks, nchunks, startgb = (
        meta["B"], meta["nblocks"], meta["nchunks"], meta["startgb"])
    tile_mech, ant_pos, ind_pos, nind = (
        meta["tile_mech"], meta["ant_pos"], meta["ind_pos"], meta["nind"])

    nc = bacc.Bacc(None, target_bir_lowering=False)
    p_xT = nc.declare_dram_parameter("xT", [F0, SHP], dt, isOutput=False)
    p_idx = [nc.declare_dram_parameter(f"idx{h}", [128, nchunks[h] * (CHUNK // 16)],
                                       mybir.dt.int16, isOutput=False) for h in (0, 1)]
    p_dl = [nc.declare_dram_parameter(f"dl{h}", [128, nblocks[h]], dt, isOutput=False)
            for h in (0, 1)]
    p_deg = nc.declare_dram_parameter("deg", [128, NT], dt, isOutput=False)
    p_W1 = nc.declare_dram_parameter("W1", [F0, F1], dt, isOutput=False)
    p_W2 = nc.declare_dram_parameter("W2", [F1, F2], dt, isOutput=False)
    p_b1 = nc.declare_dram_parameter("b1b", [128, F1], dt, isOutput=False)
    p_b2 = nc.declare_dram_parameter("b2b", [128, F2], dt, isOutput=False)
    p_iota = nc.declare_dram_parameter("iota", [128, 128], dt, isOutput=False)
    p_I = nc.declare_dram_parameter("indix", [128, max(nind, 1)], mybir.dt.int32,
                                    isOutput=False)
    p_out = nc.declare_dram_parameter("out", [128, NT * F2], dt, isOutput=True)
    import os as _os
    _dbg = bool(int(_os.environ.get("GCN_DEBUG", "0")))
    _oneblock = bool(int(_os.environ.get("GCN_ONEBLOCK", "0")))
    if _dbg:
        p_d1 = nc.declare_dram_parameter("dbg1", [128, NT * F1], dt, isOutput=True)
        p_d5 = nc.declare_dram_parameter("dbg5", [128, 128], dt, isOutput=True)
        p_d6 = nc.declare_dram_parameter("dbg6", [128, 1], dt, isOutput=True)
        p_d2 = nc.declare_dram_parameter("dbg2", [128, NT * F1], dt, isOutput=True)
        p_d3 = nc.declare_dram_parameter("dbg3", [128, F1], dt, isOutput=True)
        p_d4 = nc.declare_dram_parameter("dbg4", [128, CHUNK_BLOCKS * F1], dt, isOutput=True)

    cc_in = [nc.dram_tensor(f"cc_in{li}", [SHP, F1], dt) for li in (0, 1)]
    cc_out = [nc.dram_tensor(f"cc_out{li}", [CORES * SHP, F1], dt, addr_space="Shared")
              for li in (0, 1)]

    with tile.TileContext(nc) as tc:
        with (
            tc.tile_pool(name="cpool", bufs=1) as cpool,
            tc.tile_pool(name="spool", bufs=4) as spool,
            tc.tile_pool(name="stpool", bufs=6) as stpool,
            tc.tile_pool(name="wpool", bufs=4) as wpool,
            tc.tile_pool(name="ppool", bufs=3, space="PSUM") as ppool,
            tc.tile_pool(name="popool", bufs=2, space="PSUM") as popool,
        ):
            # ---- constants into SBUF
            xT = cpool.tile([F0, SHP], dt)
            nc.sync.dma_start(xT[:], p_xT[:])
            W1 = cpool.tile([F0, F1], dt)
            nc.sync.dma_start(W1[:], p_W1[:])
            W2 = cpool.tile([F1, F2], dt)
            nc.sync.dma_start(W2[:], p_W2[:])
            b1b = cpool.tile([128, F1], dt)
            nc.sync.dma_start(b1b[:], p_b1[:])
            b2b = cpool.tile([128, F2], dt)
            nc.sync.dma_start(b2b[:], p_b2[:])
            iota = cpool.tile([128, 128], dt)
            nc.sync.dma_start(iota[:], p_iota[:])
            degt = cpool.tile([128, NT], dt)
            nc.sync.dma_start(degt[:], p_deg[:])
            indix_sb = cpool.tile([128, max(nind, 1)], mybir.dt.int32)
            nc.sync.dma_start(indix_sb[:], p_I[:])
            idx_sb = []
            dl_sb = []
            for h in (0, 1):
                isb = cpool.tile([128, nchunks[h] * (CHUNK // 16)], mybir.dt.int16,
                                 name=f"isb{h}")
                nc.sync.dma_start(isb[:], p_idx[h][:])
                idx_sb.append(isb)
                dsb = cpool.tile([128, nblocks[h]], dt, name=f"dsb{h}")
                nc.sync.dma_start(dsb[:], p_dl[h][:])
                dl_sb.append(dsb)

            recd = cpool.tile([128, NT], dt)
            nc.vector.reciprocal(recd[:], degt[:])
            dinv = cpool.tile([128, NT], dt)
            nc.scalar.activation(dinv[:], recd[:], Act.Sqrt)

            # ---- head: T1 shard = dinv * (x @ W1)
            Tsh = cpool.tile([128, NT * F1], dt)
            for t in range(NT):
                psh = ppool.tile([128, F1], dt, tag="agg1", name=f"psh{t}")
                nc.tensor.matmul(psh[:], xT[:, BLK * t:BLK * (t + 1)], W1[:],
                                 start=True, stop=True)
                nc.vector.tensor_scalar(
                    Tsh[:, F1 * t:F1 * (t + 1)], psh[:], dinv[:, t:t + 1], None,
                    Alu.mult)
            nc.sync.dma_start(
                cc_in[0][:].rearrange("(t p) f -> p t f", p=BLK),
                Tsh.rearrange("p (t f) -> p t f", f=F1)[:])
            nc.gpsimd.collective_compute(
                "AllGather", Alu.bypass,
                ins=[cc_in[0].ap().opt()], outs=[cc_out[0].ap().opt()],
                replica_groups=[list(range(CORES))])

            def do_layer(li, table, tail_fn):
                halves = [table[0:HALF, :], table[HALF:2 * HALF, :]]
                emitted = [0, 0]
                chunks = [{}, {}]

                def ensure_chunk(h, c):
                    while emitted[h] <= min(c + 2, nchunks[h] - 1):
                        ce = emitted[h]
                        st = stpool.tile([128, CHUNK_BLOCKS, F1], dt,
                                         tag=f"st{h}", name=f"st_l{li}_h{h}_c{ce}")
                        cols = CHUNK // 16
                        nc.gpsimd.dma_gather(
                            st[:], halves[h], idx_sb[h][:, ce * cols:(ce + 1) * cols],
                            CHUNK, CHUNK, F1)
                        chunks[h][ce] = st
                        if _dbg and li == 0 and h == 0 and ce == 0:
                            nc.sync.dma_start(
                                p_d4[:], st.rearrange("p c e -> p (c e)")[:])
                        emitted[h] += 1
                    return chunks[h][c]

                for t in range(NT):
                    if li == 0:
                        pagg = ppool.tile([128, F1], dt, tag="agg1", name=f"pg{li}_{t}")
                    else:
                        pagg = ppool.tile([F1, BLK], dt, tag="agg2", name=f"pg{li}_{t}")
                    nb = int(B[t, 0] + B[t, 1])
                    if _oneblock:
                        nb = 1
                    k = 0
                    for h in ((0, 1) if not _oneblock else (0,)):
                        for b in range(int(B[t, h]) if not _oneblock else 1):
                            gb = int(startgb[t, h]) + b
                            if tile_mech[t]:
                                ic = ind_pos[h][gb]
                                sti = stpool.tile([128, F1], dt, tag="sti",
                                                  name=f"sti{li}_{t}_{h}_{b}")
                                nc.gpsimd.indirect_dma_start(
                                    sti[:], None, table[:, :],
                                    bass.IndirectOffsetOnAxis(
                                        ap=indix_sb[:, ic:ic + 1], axis=0))
                                rhs_blk = sti
                            else:
                                ap = ant_pos[h][gb]
                                c, slot = ap // CHUNK_BLOCKS, ap % CHUNK_BLOCKS
                                st = ensure_chunk(h, c)
                                rhs_blk = None
                            S = spool.tile([128, 128], dt, tag="S",
                                           name=f"S{li}_{t}_{h}_{b}")
                            nc.vector.tensor_scalar(
                                S[:], iota[:], dl_sb[h][:, gb:gb + 1], None,
                                Alu.is_equal)
                            blk_ap = (rhs_blk[:, :] if rhs_blk is not None
                                      else st[:, slot, :])
                            if li == 0:
                                nc.tensor.matmul(pagg[:], S[:], blk_ap,
                                                 start=(k == 0), stop=(k == nb - 1))
                            else:
                                nc.tensor.matmul(pagg[:], blk_ap, S[:],
                                                 start=(k == 0), stop=(k == nb - 1))
                            k += 1
                    if _dbg and li == 0 and t == 0:
                        dbg3t = wpool.tile([128, F1], dt, tag="e1", name="dbg3t")
                        nc.vector.tensor_copy(dbg3t[:], pagg[:])
                        nc.sync.dma_start(p_d3[:], dbg3t[:])
                    tail_fn(t, pagg)

            # ---- layer 1
            T2sh = cpool.tile([128, NT * F1], dt)

            def tail1(t, pagg):
                e1 = wpool.tile([128, F1], dt, tag="e1", name=f"e1_{t}")
                nc.vector.tensor_scalar(e1[:], pagg[:], dinv[:, t:t + 1], None,
                                        Alu.mult)
                e2 = wpool.tile([128, F1], dt, tag="e2", name=f"e2_{t}")
                nc.vector.tensor_tensor(out=e2[:], in0=e1[:], in1=b1b[:], op=Alu.add)
                nc.vector.tensor_scalar(
                    T2sh[:, F1 * t:F1 * (t + 1)], e2[:], 0.0, dinv[:, t:t + 1],
                    Alu.max, Alu.mult)

            do_layer(0, cc_out[0], tail1)
            nc.sync.dma_start(
                cc_in[1][:].rearrange("(t p) f -> p t f", p=BLK),
                T2sh.rearrange("p (t f) -> p t f", f=F1)[:])
            nc.gpsimd.collective_compute(
                "AllGather", Alu.bypass,
                ins=[cc_in[1].ap().opt()], outs=[cc_out[1].ap().opt()],
                replica_groups=[list(range(CORES))])

            # ---- layer 2
            outsh = cpool.tile([128, NT * F2], dt)

            def tail2(t, pagg):
                aggS = wpool.tile([F1, BLK], dt, tag="aggS", name=f"as_{t}")
                nc.vector.tensor_copy(aggS[:], pagg[:])
                po = popool.tile([128, F2], dt, tag="po", name=f"po_{t}")
                nc.tensor.matmul(po[:], aggS[:], W2[:], start=True, stop=True)
                e3 = wpool.tile([128, F2], dt, tag="e3", name=f"e3_{t}")
                nc.vector.tensor_scalar(e3[:], po[:], dinv[:, t:t + 1], None,
                                        Alu.mult)
                e4 = wpool.tile([128, F2], dt, tag="e4", name=f"e4_{t}")
                nc.vector.tensor_tensor(out=e4[:], in0=e3[:], in1=b2b[:], op=Alu.add)
                m = wpool.tile([128, 1], dt, tag="m", name=f"m_{t}")
                nc.vector.tensor_reduce(m[:], e4[:], axis=mybir.AxisListType.X,
                                        op=Alu.max)
                nm = wpool.tile([128, 1], dt, tag="nm", name=f"nm_{t}")
                nc.vector.tensor_scalar(nm[:], m[:], -1.0, None, Alu.mult)
                ex = wpool.tile([128, F2], dt, tag="ex", name=f"ex_{t}")
                nc.scalar.activation(ex[:], e4[:], Act.Exp, bias=nm[:, 0:1])
                sm = wpool.tile([128, 1], dt, tag="sm", name=f"sm_{t}")
                nc.vector.tensor_reduce(sm[:], ex[:], axis=mybir.AxisListType.X,
                                        op=Alu.add)
                lg = wpool.tile([128, 1], dt, tag="lg", name=f"lg_{t}")
                nc.scalar.activation(lg[:], sm[:], Act.Ln)
                nc.vector.tensor_scalar(
                    outsh[:, F2 * t:F2 * (t + 1)], e4[:], m[:, 0:1], lg[:, 0:1],
                    Alu.subtract, Alu.subtract)

            do_layer(1, cc_out[1], tail2)
            nc.sync.dma_start(p_out[:], outsh[:])
            if _dbg:
                nc.sync.dma_start(p_d1[:], Tsh[:])
                nc.sync.dma_start(p_d2[:], T2sh[:])

    nc.finalize()
    return nc


LAST_EXEC_NS = None


def kernel(x, edge_index, W1, b1, W2, b2):
    from concourse.bass_utils import run_bass_kernel_spmd

    x = np.asarray(x, np.float32)
    data, consts, meta = host_prep(x, np.asarray(edge_index), W1, b1, W2, b2)
    nc = build_nc(meta)
    in_maps = []
    for i in range(CORES):
        m = dict(data[i])
        m.update({k: np.ascontiguousarray(v) for k, v in consts.items()})
        in_maps.append(m)
    import os as _os
    trace = bool(int(_os.environ.get("GCN_TRACE", "0")))
    res = run_bass_kernel_spmd(nc, in_maps, core_ids=list(range(CORES)), trace=trace)
    global LAST_EXEC_NS
    LAST_EXEC_NS = res.exec_time_ns
    if trace and res.instructions_and_trace:
        try:
            import pickle
            insts, tpath = res.instructions_and_trace
            with open("/tmp/gcn_insts.pkl", "wb") as f:
                pickle.dump({"insts": insts, "exec_ns": res.exec_time_ns,
                             "trace_path": tpath}, f)
        except Exception as e:
            print("trace stash failed:", e)
    outs = []
    for i in range(CORES):
        o = res.results[i]["out"]  # [128, NT*F2]
        outs.append(o.reshape(128, NT, F2).transpose(1, 0, 2).reshape(SHP, F2))
    return assemble_output(np.stack(outs))


if __name__ == "__main__":
    import reference
    inputs = {k: np.asarray(v) for k, v in reference.setup_inputs().items()}
    expected = np.asarray(reference.reference(**{k: v for k, v in inputs.items()}))
    data, consts, meta = host_prep(**inputs)
    print("nblocks:", meta["nblocks"], "nchunks:", meta["nchunks"])
    outs = numpy_device_sim(data, consts, meta)
    got = assemble_output(outs)
    err = np.abs(got - expected)
    rel = err.max() / np.abs(expected).max()
    print(f"numpy-sim max abs err {err.max():.3e}  rel {rel:.3e}")



# revision 5
# speedup vs baseline: 1.4310x; 1.4310x over previous
"""GCN (2-layer) Trainium2 kernel over 8 NeuronCores — v2.

Strategy (dst-sharded pull-gather, bf16):
- Nodes sharded contiguously: core i owns nodes [6250*i, 6250*(i+1)).
- Layer table rows are bf16, PADDED to 128 values (64 real + 64 unread) so
  every row is one 256-byte gather element (dma_gather requires 256B-aligned
  elements; gather cost is per-descriptor latency, so padding is free).
- The table is AllGather'd in two halves (src tiles 0:25 / 25:49) so the
  second half's collective overlaps the first half's gather+aggregate pass.
- Aggregation per 128-dst tile: gathered 128-edge blocks are summed with a
  TensorE matmul against a DVE-built one-hot selector S (bf16).
- Self-loop terms are folded into the per-tile tail math (never gathered).
- Gathers run in 2048-index chunks round-robined over 4 SWDGE queues.
- Layer 2 aggregates the (dinv*relu(.)) table, then applies W2 after a PE
  transpose; log_softmax on ACT+DVE.
The edge structure is baked into the program; block counts are padded to the
max over cores so the SPMD program is identical on all 8 cores.
"""

import numpy as np

try:
    from ml_dtypes import bfloat16 as bf16np
except ImportError:  # pragma: no cover
    bf16np = None

N_NODES = 50000
CORES = 8
SH = 6250          # owned nodes per core
SHP = 6272         # padded shard rows (49*128)
NT = 49            # dst tiles per core
BLK = 128
F0, F1, F2 = 96, 64, 16
TA = 25            # tiles in half A
TB = NT - TA       # 24
WA, WB = TA * BLK, TB * BLK          # padded cols per half: 3200 / 3072
EA, EB = CORES * WA, CORES * WB      # table elements per half: 25600 / 24576
CHUNK = 2048
CB = CHUNK // BLK  # blocks per chunk
NQ = 4             # SWDGE queues
PAD_DL = 300.0     # is_equal miss => zero S row
LOOKAHEAD = 6


def _bf(x):
    return np.asarray(x, np.float32).astype(bf16np)


def host_prep(x, edge_index, W1, b1, W2, b2):
    src = np.asarray(edge_index[0], dtype=np.int64)
    dst = np.asarray(edge_index[1], dtype=np.int64)
    deg_full = np.bincount(dst, minlength=N_NODES).astype(np.float32) + 1.0

    # split edges by dst shard
    order = np.argsort(dst, kind="stable")
    s_sorted, d_sorted = src[order], dst[order]
    bounds = np.searchsorted(d_sorted, np.arange(0, N_NODES + 1, SH))

    # per-core, per-pass, per-tile edge lists: (elem, dl)
    counts = np.zeros((CORES, 2, NT), np.int64)
    lists = [[[None] * NT for _ in range(2)] for _ in range(CORES)]
    for i in range(CORES):
        es = s_sorted[bounds[i]:bounds[i + 1]]
        ed = d_sorted[bounds[i]:bounds[i + 1]] - SH * i
        s_sh = es // SH
        l = es - SH * s_sh
        ts = l // BLK
        p = l - ts * BLK
        half = (ts >= TA).astype(np.int64)
        elem = np.where(half == 0,
                        s_sh * WA + p * TA + ts,
                        s_sh * WB + p * TB + (ts - TA))
        t = ed // BLK
        dl = ed - t * BLK
        key = half * NT + t
        o = np.argsort(key, kind="stable")
        key_s, elem_s, dl_s = key[o], elem[o], dl[o]
        kb = np.searchsorted(key_s, np.arange(2 * NT + 1))
        for P in range(2):
            for tt in range(NT):
                a, b = kb[P * NT + tt], kb[P * NT + tt + 1]
                lists[i][P][tt] = (elem_s[a:b], dl_s[a:b])
                counts[i, P, tt] = b - a

    # uniform block counts across cores
    B = np.maximum(1, -(-counts.max(axis=0) // BLK))  # [2, NT]
    nblk = [int(B[P].sum()) for P in range(2)]
    nch = [-(-nblk[P] // CB) for P in range(2)]

    data = []
    for i in range(CORES):
        d = dict()
        for P in range(2):
            els, dls = [], []
            for tt in range(NT):
                e, q = lists[i][P][tt]
                pad = int(B[P, tt]) * BLK - len(e)
                els.append(np.concatenate([e, np.zeros(pad, np.int64)]))
                dls.append(np.concatenate([q.astype(np.float32),
                                           np.full(pad, PAD_DL, np.float32)]))
            estream = np.concatenate(els)
            dstream = np.concatenate(dls)
            tail = nch[P] * CHUNK - len(estream)
            estream = np.concatenate([estream, np.zeros(tail, np.int64)])
            # idx plane [128, nch*128]: idx j at [j%16, j//16], replicated x8
            pl = estream.reshape(-1, 16).T.astype(np.int16)
            d[f"idx{P}"] = np.ascontiguousarray(np.tile(pl, (8, 1)))
            # dl plane [128, nblk]
            d[f"dl{P}"] = np.ascontiguousarray(
                dstream.reshape(-1, BLK).T.astype(np.float32))
        degp = np.ones((BLK, NT), np.float32)
        dsh = deg_full[SH * i:SH * (i + 1)]
        dp = np.concatenate([dsh, np.ones(SHP - SH, np.float32)])
        degp[:, :] = dp.reshape(NT, BLK).T
        d["deg"] = np.ascontiguousarray(degp)
        xs = np.zeros((F0, SHP), np.float32)
        xs[:, :SH] = np.asarray(x[SH * i:SH * (i + 1)], np.float32).T
        d["xT"] = np.ascontiguousarray(_bf(xs))
        data.append(d)

    ident = np.eye(BLK, dtype=np.float32)
    consts = dict(
        W1=_bf(W1), W2=_bf(W2),
        b1b=np.tile(np.asarray(b1, np.float32), (BLK, 1)),
        b2b=np.tile(np.asarray(b2, np.float32), (BLK, 1)),
        iota=_bf(np.tile(np.arange(BLK, dtype=np.float32), (BLK, 1))),
        ident=ident,
    )
    meta = dict(B=B, nblk=nblk, nch=nch)
    return data, consts, meta


def numpy_sim(x, edge_index, W1, b1, W2, b2):
    """Emulate the device numerics (bf16 tables/weights) edge-wise."""
    def f(a):
        return _bf(a).astype(np.float32)

    src = np.asarray(edge_index[0]); dst = np.asarray(edge_index[1])
    deg = np.bincount(dst, minlength=N_NODES).astype(np.float32) + 1.0
    dinv = 1.0 / np.sqrt(deg)
    h1 = f(x) @ f(W1)  # f32 accum of bf16 operands
    TshF = dinv[:, None] * h1
    table1 = f(TshF)
    G1 = np.zeros_like(TshF)
    np.add.at(G1, dst, table1[src])
    e1 = dinv[:, None] * (G1 + TshF) + np.asarray(b1, np.float32)
    T2F = dinv[:, None] * np.maximum(e1, 0.0)
    table2 = f(T2F)
    G2 = np.zeros_like(T2F)
    np.add.at(G2, dst, table2[src])
    vs = dinv[:, None] * (G2 + T2F)
    z = f(vs) @ f(W2) + np.asarray(b2, np.float32)
    m = z.max(1, keepdims=True)
    return z - m - np.log(np.exp(z - m).sum(1, keepdims=True))


def build_nc(meta):
    import concourse.bacc as bacc
    import concourse.tile as tile
    import concourse.mybir as mybir

    dt = mybir.dt.float32
    bf = mybir.dt.bfloat16
    Alu = mybir.AluOpType
    Act = mybir.ActivationFunctionType
    B, nblk, nch = meta["B"], meta["nblk"], meta["nch"]

    nc = bacc.Bacc(None, target_bir_lowering=False, num_swdge_queues=NQ)
    p_xT = nc.declare_dram_parameter("xT", [F0, SHP], bf, isOutput=False)
    p_idx = [nc.declare_dram_parameter(f"idx{P}", [128, nch[P] * (CHUNK // 16)],
                                       mybir.dt.int16, isOutput=False)
             for P in range(2)]
    p_dl = [nc.declare_dram_parameter(f"dl{P}", [128, nblk[P]], dt,
                                      isOutput=False) for P in range(2)]
    p_deg = nc.declare_dram_parameter("deg", [128, NT], dt, isOutput=False)
    p_W1 = nc.declare_dram_parameter("W1", [F0, F1], bf, isOutput=False)
    p_W2 = nc.declare_dram_parameter("W2", [F1, F2], bf, isOutput=False)
    p_b1 = nc.declare_dram_parameter("b1b", [128, F1], dt, isOutput=False)
    p_b2 = nc.declare_dram_parameter("b2b", [128, F2], dt, isOutput=False)
    p_iota = nc.declare_dram_parameter("iota", [128, 128], bf, isOutput=False)
    p_ident = nc.declare_dram_parameter("ident", [128, 128], dt, isOutput=False)
    p_out = nc.declare_dram_parameter("out", [128, NT * F2], dt, isOutput=True)

    cc_in = [[nc.dram_tensor(f"cc_in{li}{P}", [128, (WA, WB)[P]], bf)
              for P in range(2)] for li in range(2)]
    cc_out = [[nc.dram_tensor(f"cc_out{li}{P}", [(EA, EB)[P], 128], bf,
                              addr_space="Shared")
               for P in range(2)] for li in range(2)]

    with tile.TileContext(nc) as tc:
        with (
            tc.tile_pool(name="cpool", bufs=1) as cpool,
            tc.tile_pool(name="spool", bufs=8) as spool,
            tc.tile_pool(name="stpool", bufs=8) as stpool,
            tc.tile_pool(name="wpool", bufs=3) as wpool,
            tc.tile_pool(name="ppool", bufs=4, space="PSUM") as ppool,
            tc.tile_pool(name="p2pool", bufs=2, space="PSUM") as p2pool,
        ):
            # ---- constants into SBUF
            xT = cpool.tile([F0, SHP], bf)
            nc.sync.dma_start(xT[:], p_xT[:])
            W1 = cpool.tile([F0, F1], bf)
            nc.sync.dma_start(W1[:], p_W1[:])
            W2 = cpool.tile([F1, F2], bf)
            nc.sync.dma_start(W2[:], p_W2[:])
            b1b = cpool.tile([128, F1], dt)
            nc.sync.dma_start(b1b[:], p_b1[:])
            b2b = cpool.tile([128, F2], dt)
            nc.sync.dma_start(b2b[:], p_b2[:])
            iota = cpool.tile([128, 128], bf)
            nc.sync.dma_start(iota[:], p_iota[:])
            ident = cpool.tile([128, 128], dt)
            nc.sync.dma_start(ident[:], p_ident[:])
            degt = cpool.tile([128, NT], dt)
            nc.sync.dma_start(degt[:], p_deg[:])
            idx_sb = []
            dl_sb = []
            for P in range(2):
                isb = cpool.tile([128, nch[P] * (CHUNK // 16)], mybir.dt.int16,
                                 name=f"isb{P}")
                nc.sync.dma_start(isb[:], p_idx[P][:])
                idx_sb.append(isb)
                dsb = cpool.tile([128, nblk[P]], dt, name=f"dsb{P}")
                nc.sync.dma_start(dsb[:], p_dl[P][:])
                dl_sb.append(dsb)

            recd = cpool.tile([128, NT], dt)
            nc.vector.reciprocal(recd[:], degt[:])
            dinv = cpool.tile([128, NT], dt)
            nc.scalar.activation(dinv[:], recd[:], Act.Sqrt)

            TshF = cpool.tile([128, NT * F1], dt)
            Tpad = cpool.tile([128, NT * BLK], bf)
            T2F = cpool.tile([128, NT * F1], dt)
            T2pad = cpool.tile([128, NT * BLK], bf)
            accA = cpool.tile([128, NT * F1], dt)
            outsh = cpool.tile([128, NT * F2], dt)

            def fire_ag(li, P, pad_src):
                w0 = 0 if P == 0 else WA
                w1 = WA if P == 0 else WA + WB
                nc.sync.dma_start(cc_in[li][P][:], pad_src[:, w0:w1])
                nc.gpsimd.collective_compute(
                    "AllGather", Alu.bypass,
                    ins=[cc_in[li][P].ap().opt()],
                    outs=[cc_out[li][P].ap().opt()],
                    replica_groups=[list(range(CORES))])

            # ---- head: TshF = dinv * (x @ W1); Tpad gets bf16 copy
            for t in range(NT):
                psh = ppool.tile([128, F1], dt, tag="agg", name=f"psh{t}")
                nc.tensor.matmul(psh[:], xT[:, BLK * t:BLK * (t + 1)], W1[:],
                                 start=True, stop=True)
                nc.vector.tensor_scalar(
                    TshF[:, F1 * t:F1 * (t + 1)], psh[:], dinv[:, t:t + 1],
                    None, Alu.mult)
                nc.scalar.copy(Tpad[:, BLK * t:BLK * t + F1],
                               TshF[:, F1 * t:F1 * (t + 1)])
                if t == TA - 1:
                    fire_ag(0, 0, Tpad)
            fire_ag(0, 1, Tpad)

            qcounter = [0]

            def do_pass(li, P, tail_fn):
                table = cc_out[li][P]
                emitted = [0]
                chunks = {}

                def ensure_chunk(c):
                    while emitted[0] <= min(c + LOOKAHEAD, nch[P] - 1):
                        ce = emitted[0]
                        st = stpool.tile([128, CB, 128], bf, tag="st",
                                         name=f"st_l{li}p{P}c{ce}")
                        cols = CHUNK // 16
                        nc.gpsimd.dma_gather(
                            st[:], table[:],
                            idx_sb[P][:, ce * cols:(ce + 1) * cols],
                            CHUNK, CHUNK, 128,
                            single_packet=False,
                            queue_num=qcounter[0] % NQ)
                        qcounter[0] += 1
                        chunks[ce] = st
                        if ce >= LOOKAHEAD + 1:
                            chunks.pop(ce - LOOKAHEAD - 1, None)
                        emitted[0] += 1
                    return chunks[c]

                gb = 0
                for t in range(NT):
                    nb = int(B[P, t])
                    pagg = ppool.tile([128, F1], dt, tag="agg",
                                      name=f"pg{li}{P}_{t}")
                    for b in range(nb):
                        c, slot = gb // CB, gb % CB
                        st = ensure_chunk(c)
                        S = spool.tile([128, 128], bf, tag="S",
                                       name=f"S{li}{P}_{gb}")
                        nc.vector.tensor_scalar(
                            S[:], iota[:], dl_sb[P][:, gb:gb + 1], None,
                            Alu.is_equal)
                        nc.tensor.matmul(pagg[:], S[:], st[:, slot, 0:F1],
                                         start=(b == 0), stop=(b == nb - 1))
                        gb += 1
                    tail_fn(t, pagg)

            # ---- layer 1 pass A: spill
            def spillA(t, pagg):
                nc.scalar.copy(accA[:, F1 * t:F1 * (t + 1)], pagg[:])

            do_pass(0, 0, spillA)

            # ---- layer 1 pass B: tail computes T2
            def tail1(t, pagg):
                u = wpool.tile([128, F1], dt, tag="u", name=f"u1_{t}")
                nc.vector.tensor_tensor(
                    out=u[:], in0=pagg[:], in1=accA[:, F1 * t:F1 * (t + 1)],
                    op=Alu.add)
                v = wpool.tile([128, F1], dt, tag="v", name=f"v1_{t}")
                nc.vector.tensor_tensor(
                    out=v[:], in0=u[:], in1=TshF[:, F1 * t:F1 * (t + 1)],
                    op=Alu.add)
                e1 = wpool.tile([128, F1], dt, tag="e1", name=f"e1_{t}")
                nc.vector.scalar_tensor_tensor(
                    e1[:], v[:], dinv[:, t:t + 1], b1b[:],
                    Alu.mult, Alu.add)
                nc.vector.tensor_scalar(
                    T2F[:, F1 * t:F1 * (t + 1)], e1[:], 0.0, dinv[:, t:t + 1],
                    Alu.max, Alu.mult)
                nc.scalar.copy(T2pad[:, BLK * t:BLK * t + F1],
                               T2F[:, F1 * t:F1 * (t + 1)])
                if t == TA - 1:
                    fire_ag(1, 0, T2pad)
                if t == NT - 1:
                    fire_ag(1, 1, T2pad)

            do_pass(0, 1, tail1)

            # ---- layer 2 pass A
            do_pass(1, 0, spillA)

            # ---- layer 2 pass B: transpose, W2, log_softmax
            def tail2(t, pagg):
                u = wpool.tile([128, F1], dt, tag="u", name=f"u2_{t}")
                nc.vector.tensor_tensor(
                    out=u[:], in0=pagg[:], in1=accA[:, F1 * t:F1 * (t + 1)],
                    op=Alu.add)
                vs = wpool.tile([128, F1], dt, tag="v", name=f"vs_{t}")
                nc.vector.scalar_tensor_tensor(
                    vs[:], u[:], 1.0, T2F[:, F1 * t:F1 * (t + 1)],
                    Alu.mult, Alu.add)
                vsc = wpool.tile([128, F1], dt, tag="vsc", name=f"vsc_{t}")
                nc.vector.tensor_scalar(
                    vsc[:], vs[:], dinv[:, t:t + 1], None, Alu.mult)
                trp = p2pool.tile([F1, 128], dt, tag="tr", name=f"tr_{t}")
                nc.tensor.transpose(trp[:], vsc[:], ident[:])
                zT = wpool.tile([F1, 128], bf, tag="zT", name=f"zT_{t}")
                nc.scalar.copy(zT[:], trp[:])
                po = p2pool.tile([128, F2], dt, tag="po", name=f"po_{t}")
                nc.tensor.matmul(po[:], zT[:], W2[:], start=True, stop=True)
                e4 = wpool.tile([128, F2], dt, tag="e4", name=f"e4_{t}")
                nc.vector.tensor_tensor(out=e4[:], in0=po[:], in1=b2b[:],
                                        op=Alu.add)
                m = wpool.tile([128, 1], dt, tag="m", name=f"m_{t}")
                nc.vector.tensor_reduce(m[:], e4[:], axis=mybir.AxisListType.X,
                                        op=Alu.max)
                nm = wpool.tile([128, 1], dt, tag="nm", name=f"nm_{t}")
                nc.vector.tensor_scalar(nm[:], m[:], -1.0, None, Alu.mult)
                ex = wpool.tile([128, F2], dt, tag="ex", name=f"ex_{t}")
                nc.scalar.activation(ex[:], e4[:], Act.Exp, bias=nm[:, 0:1])
                sm = wpool.tile([128, 1], dt, tag="sm", name=f"sm_{t}")
                nc.vector.tensor_reduce(sm[:], ex[:], axis=mybir.AxisListType.X,
                                        op=Alu.add)
                lg = wpool.tile([128, 1], dt, tag="lg", name=f"lg_{t}")
                nc.scalar.activation(lg[:], sm[:], Act.Ln)
                nc.vector.tensor_scalar(
                    outsh[:, F2 * t:F2 * (t + 1)], e4[:], m[:, 0:1],
                    lg[:, 0:1], Alu.subtract, Alu.subtract)

            do_pass(1, 1, tail2)
            nc.sync.dma_start(p_out[:], outsh[:])

    nc.finalize()
    return nc


LAST_EXEC_NS = None


def kernel(x, edge_index, W1, b1, W2, b2):
    from concourse.bass_utils import run_bass_kernel_spmd

    x = np.asarray(x, np.float32)
    data, consts, meta = host_prep(x, np.asarray(edge_index), W1, b1, W2, b2)
    nc = build_nc(meta)
    in_maps = []
    for i in range(CORES):
        m = dict(data[i])
        m.update({k: np.ascontiguousarray(v) for k, v in consts.items()})
        in_maps.append(m)
    import os as _os
    trace = bool(int(_os.environ.get("GCN_TRACE", "0")))
    res = run_bass_kernel_spmd(nc, in_maps, core_ids=list(range(CORES)),
                               trace=trace)
    global LAST_EXEC_NS
    LAST_EXEC_NS = res.exec_time_ns
    if trace and res.instructions_and_trace:
        try:
            import pickle
            insts, tpath = res.instructions_and_trace
            with open("/tmp/gcn_insts.pkl", "wb") as f:
                pickle.dump({"insts": insts, "exec_ns": res.exec_time_ns,
                             "trace_path": tpath}, f)
        except Exception as e:
            print("trace stash failed:", e)
    outs = []
    for i in range(CORES):
        o = res.results[i]["out"]  # [128, NT*F2]
        outs.append(o.reshape(128, NT, F2).transpose(1, 0, 2).reshape(SHP, F2))
    res_full = np.zeros((N_NODES, F2), np.float32)
    for i in range(CORES):
        res_full[SH * i:SH * (i + 1)] = outs[i][:SH]
    return res_full


if __name__ == "__main__":
    z = np.load("/tmp/gcn_ref.npz")
    inputs = {k: z[k] for k in z.files if k != "expected"}
    expected = z["expected"]
    data, consts, meta = host_prep(**inputs)
    print("nblk:", meta["nblk"], "nch:", meta["nch"])
    got = numpy_sim(**inputs)
    err = np.abs(got - expected)
    rel = err.max() / np.abs(expected).max()
    print(f"numpy-sim (bf16 emul) max abs err {err.max():.3e}  rel {rel:.3e}")
